# revision 11
# baseline (speedup 1.0000x reference)
"""MultiHeadAttention Trainium2 kernel: B=8, S=1024, D=1024, H=16, DK=64.

Batch-parallel over 8 NeuronCores (one batch element per core, no
collectives). Per core, attention runs in a transposed layout
(scores^T[s,q]) so the attn@V contraction needs no on-device
transposes; all data marshaling (transposes, weight reshape, mask
preprocessing) happens host-side in numpy.

Two variants:
 - structured: mask0[b] == broadcast(valid), mask1[b] == outer(valid,valid)
   (what reference.setup_inputs produces). mask0 folds into the exp bias
   ([P,1] per s-tile), mask1's row factor folds into the final scale, and
   the softmax denominator rides as 64 `ones` columns inside the V lhsT.
 - general: arbitrary 0/1 masks. mask0 is added as a -30000 bias via an
   identity matmul into the scores PSUM; mask1 multiplies e_d on the DVE;
   denominators use separate ones-matmuls.
"""
import os
import numpy as np
import ml_dtypes

import concourse.bacc as bacc
import concourse.mybir as mybir
from concourse.tile import TileContext
from concourse.bass_utils import run_bass_kernel_spmd

B, S, D, H, DK = 8, 1024, 1024, 16, 64
NP = 8
f32 = mybir.dt.float32
f32r = mybir.dt.float32r
bf16 = mybir.dt.bfloat16
AF = mybir.ActivationFunctionType
ALU = mybir.AluOpType


def _build(structured: bool):
    nc = bacc.Bacc("TRN2", target_bir_lowering=False, debug=False, num_devices=8)
    dt_qk = f32r if structured else bf16

    xqt_d = nc.dram_tensor("xqt", [D, S], f32, kind="ExternalInput")
    xkt_d = nc.dram_tensor("xkt", [D, S], f32, kind="ExternalInput")
    xvt_d = nc.dram_tensor("xvt", [D, S], f32, kind="ExternalInput")
    w2q_d = nc.dram_tensor("w2q", [D, D], f32, kind="ExternalInput")
    w2k_d = nc.dram_tensor("w2k", [D, D], f32, kind="ExternalInput")
    w2v_d = nc.dram_tensor("w2v", [D, D], f32, kind="ExternalInput")
    wo_d = nc.dram_tensor("wo", [D, D], f32, kind="ExternalInput")
    bq8_d = nc.dram_tensor("bq8", [D], f32, kind="ExternalInput")
    bk_d = nc.dram_tensor("bk", [D], f32, kind="ExternalInput")
    bv_d = nc.dram_tensor("bv", [D], f32, kind="ExternalInput")
    bo_d = nc.dram_tensor("bo_bc", [128, D], f32, kind="ExternalInput")
    onescol_d = nc.dram_tensor("onescol", [128, 8 * 64], f32, kind="ExternalInput")
    zeros_d = nc.dram_tensor("zeros", [64, S], f32, kind="ExternalInput")
    if structured:
        expb_d = nc.dram_tensor("expb", [128, NP], f32, kind="ExternalInput")
        vbq_d = nc.dram_tensor("vbq", [128, S], f32, kind="ExternalInput")
        vq8_d = nc.dram_tensor("vq8", [128, NP], f32, kind="ExternalInput")
    else:
        mbT_d = nc.dram_tensor("mbT", [S, S], bf16, kind="ExternalInput")
        m1T_d = nc.dram_tensor("m1T", [S, S], f32, kind="ExternalInput")
        eye_d = nc.dram_tensor("eye", [128, 128], bf16, kind="ExternalInput")
    y_d = nc.dram_tensor("y", [S, D], f32, kind="ExternalOutput")

    with TileContext(nc) as tc, \
         tc.tile_pool(name="persist", bufs=1) as app, \
         tc.tile_pool(name="vecs", bufs=1) as vp:
        aot = [app.tile([128, S], f32r, tag=f"aot{j}", name=f"aot{j}")
               for j in range(NP)]
        bq8v = vp.tile([128, NP], f32, name="bq8v")
        bkv = vp.tile([128, NP], f32, name="bkv")
        bvv = vp.tile([128, NP], f32, name="bvv")
        bob = vp.tile([128, D], f32, name="bob")
        nc.sync.dma_start(bq8v[:], bq8_d.ap().rearrange("(t p) -> p t", p=128))
        nc.sync.dma_start(bkv[:], bk_d.ap().rearrange("(t p) -> p t", p=128))
        nc.sync.dma_start(bvv[:], bv_d.ap().rearrange("(t p) -> p t", p=128))
        nc.sync.dma_start(bob[:], bo_d.ap())
        onesrow = vp.tile([1, 128], f32r, name="onesrow")
        bvr = vp.tile([1, D], f32r, name="bvr")
        nc.sync.dma_start(onesrow[:], onescol_d.ap()[0:1, 0:128].bitcast(f32r))
        nc.sync.dma_start(bvr[:],
                          bv_d.ap().rearrange("(a d) -> a d", a=1).bitcast(f32r))
        if structured:
            expb = vp.tile([128, NP], f32, name="expb")
            vbq = vp.tile([128, S], f32, name="vbq")
            vq8 = vp.tile([128, NP], f32, name="vq8")
            nc.sync.dma_start(expb[:], expb_d.ap())
            nc.sync.dma_start(vbq[:], vbq_d.ap())
            nc.sync.dma_start(vq8[:], vq8_d.ap())

        with tc.tile_pool(name="qkv", bufs=1) as pp:
            qtz = [pp.tile([128, S], dt_qk, tag=f"qtz{h}", name=f"qtz{h}")
                   for h in range(H)]
            kt = [pp.tile([128, S], dt_qk, tag=f"kt{p}", name=f"kt{p}")
                  for p in range(NP)]
            if structured:
                vpack = [pp.tile([128, NP, 192], f32r, tag=f"vp{j}",
                                 name=f"vp{j}") for j in range(NP)]
            else:
                vpack = [pp.tile([128, S], f32r, tag=f"vp{j}", name=f"vp{j}")
                         for j in range(NP)]
                ones64 = pp.tile([128, 64], f32r, name="ones64")
                nc.sync.dma_start(ones64[:],
                                  onescol_d.ap()[:, 0:64].bitcast(f32r))
                mbT = [pp.tile([128, S], bf16, tag=f"mbT{j}", name=f"mbT{j}")
                       for j in range(NP)]
                m1T = [pp.tile([128, S], f32, tag=f"m1T{j}", name=f"m1T{j}")
                       for j in range(NP)]
                eye = pp.tile([128, 128], bf16, name="eye")
                nc.sync.dma_start(eye[:], eye_d.ap())
                for j in range(NP):
                    nc.sync.dma_start(mbT[j][:],
                                      mbT_d.ap()[j * 128:(j + 1) * 128, :])
                    nc.sync.dma_start(m1T[j][:],
                                      m1T_d.ap()[j * 128:(j + 1) * 128, :])

            # ---------------- PHASE A: projections ----------------
            with tc.tile_pool(name="xt", bufs=2) as xp, \
                 tc.tile_pool(name="wst", bufs=6) as wp, \
                 tc.tile_pool(name="wvp", bufs=2) as wvp, \
                 tc.tile_pool(name="pjp", bufs=1, space="PSUM") as pjp:
                for which, (xd, wd) in enumerate(
                    [(xqt_d, w2q_d), (xkt_d, w2k_d), (xvt_d, w2v_d)]
                ):
                    for half in range(2):
                        js = list(range(half * 4, half * 4 + 4))
                        pss = {}
                        for j in js:
                            pss[j] = pjp.tile([128, S], f32, tag=f"pj{j % 4}",
                                              name=f"pj{which}_{j}")
                        for i in range(NP):
                            x = xp.tile([128, S], f32r, tag="x",
                                        name=f"x{which}_{half}_{i}")
                            nc.sync.dma_start(
                                x[:],
                                xd.ap()[i * 128:(i + 1) * 128, :].bitcast(f32r))
                            if which == 2:
                                # V natural [s, n]: lhsT = XvT d-tile sliced
                                # per s-tile, rhs = W2v row-tile
                                wv = wvp.tile([128, S], f32r, tag="wv",
                                             name=f"wv{half}_{i}")
                                nc.sync.dma_start(
                                    wv[:],
                                    wd.ap()[i * 128:(i + 1) * 128,
                                            :].bitcast(f32r))
                                for j in js:
                                    for c in range(2):
                                        nc.tensor.matmul(
                                            pss[j][:, c * 512:(c + 1) * 512],
                                            x[:, j * 128:(j + 1) * 128],
                                            wv[:, c * 512:(c + 1) * 512],
                                            start=(i == 0), stop=False)
                                if i == NP - 1:
                                    for j in js:
                                        for c in range(2):
                                            nc.tensor.matmul(
                                                pss[j][:, c * 512:(c + 1) * 512],
                                                onesrow[:],
                                                bvr[:, c * 512:(c + 1) * 512],
                                                start=False, stop=True)
                                continue
                            for j in js:
                                w = wp.tile([128, 128], f32r, tag="w",
                                            name=f"w{which}_{j}_{i}")
                                nc.sync.dma_start(
                                    w[:],
                                    wd.ap()[i * 128:(i + 1) * 128,
                                            j * 128:(j + 1) * 128].bitcast(f32r))
                                for c in range(2):
                                    nc.tensor.matmul(
                                        pss[j][:, c * 512:(c + 1) * 512], w[:],
                                        x[:, c * 512:(c + 1) * 512],
                                        start=(i == 0), stop=(i == NP - 1))
                        for j in js:
                            ps = pss[j]
                            if which == 0:
                                nc.vector.tensor_scalar(
                                    out=qtz[2 * j][0:64, :], in0=ps[0:64, :],
                                    scalar1=0.125, scalar2=bq8v[0:64, j:j + 1],
                                    op0=ALU.mult, op1=ALU.add)
                                nc.vector.tensor_scalar(
                                    out=qtz[2 * j + 1][64:128, :],
                                    in0=ps[64:128, :],
                                    scalar1=0.125, scalar2=bq8v[64:128, j:j + 1],
                                    op0=ALU.mult, op1=ALU.add)
                                nc.sync.dma_start(
                                    qtz[2 * j][64:128, :],
                                    zeros_d.ap().bitcast(f32r))
                                nc.sync.dma_start(
                                    qtz[2 * j + 1][0:64, :],
                                    zeros_d.ap().bitcast(f32r))
                            elif which == 1:
                                nc.vector.tensor_scalar(
                                    out=kt[j][:], in0=ps[:],
                                    scalar1=bkv[:, j:j + 1], scalar2=None,
                                    op0=ALU.add)
                            else:
                                if structured:
                                    nc.vector.tensor_copy(
                                        vpack[j][:, :, 0:64],
                                        ps[:].rearrange(
                                            "p (a two c) -> p a two c",
                                            two=2, c=64)[:, :, 0, :])
                                    nc.vector.tensor_copy(
                                        vpack[j][:, :, 128:192],
                                        ps[:].rearrange(
                                            "p (a two c) -> p a two c",
                                            two=2, c=64)[:, :, 1, :])
                                    nc.sync.dma_start(
                                        vpack[j][:, :, 64:128],
                                        onescol_d.ap().rearrange(
                                            "p (a c) -> p a c",
                                            c=64).bitcast(f32r))
                                else:
                                    nc.vector.tensor_copy(vpack[j][:], ps[:])

            # ---------------- PHASE B: attention ----------------
            with tc.tile_pool(name="ed", bufs=2) as ep, \
                 tc.tile_pool(name="epi", bufs=1) as wk, \
                 tc.tile_pool(name="scp", bufs=2, space="PSUM") as scp, \
                 tc.tile_pool(name="nvp", bufs=1, space="PSUM") as nvp:
                for p in range(NP):
                    hA, hB = 2 * p, 2 * p + 1
                    for qc in range(2):
                        qs = slice(qc * 512, (qc + 1) * 512)
                        psA = nvp.tile([128, 512], f32, tag="psA",
                                       name=f"psA{p}_{qc}")
                        psB = nvp.tile([128, 512], f32, tag="psB",
                                       name=f"psB{p}_{qc}")
                        if not structured:
                            dAB = nvp.tile([128, 512], f32, tag="dAB",
                                           name=f"dAB{p}_{qc}")
                        for i in range(NP):
                            sc = scp.tile([128, 1024], f32, tag="sc",
                                          name=f"sc{p}_{qc}_{i}")
                            nc.tensor.matmul(
                                sc[:, 0:512], kt[p][:, i * 128:(i + 1) * 128],
                                qtz[hA][:, qs], start=True, stop=structured,
                                skip_group_check=True)
                            nc.tensor.matmul(
                                sc[:, 512:1024], kt[p][:, i * 128:(i + 1) * 128],
                                qtz[hB][:, qs], start=True, stop=structured,
                                skip_group_check=True)
                            if not structured:
                                nc.tensor.matmul(
                                    sc[:, 0:512], eye[:], mbT[i][:, qs],
                                    start=False, stop=True,
                                    skip_group_check=True)
                                nc.tensor.matmul(
                                    sc[:, 512:1024], eye[:], mbT[i][:, qs],
                                    start=False, stop=True,
                                    skip_group_check=True)
                            e_d = ep.tile([128, 1024], f32r, tag="ed",
                                          name=f"ed{p}_{qc}_{i}")
                            if structured:
                                nc.scalar.activation(
                                    e_d[:], sc[:], AF.Exp,
                                    bias=expb[:, i:i + 1], scale=1.0)
                                nc.tensor.matmul(
                                    psA[:], vpack[i][:, p, 0:128],
                                    e_d[:, 0:512],
                                    start=(i == 0), stop=(i == NP - 1),
                                    skip_group_check=True)
                                nc.tensor.matmul(
                                    psB[:], vpack[i][:, p, 64:192],
                                    e_d[:, 512:1024],
                                    start=(i == 0), stop=(i == NP - 1),
                                    skip_group_check=True)
                            else:
                                nc.scalar.activation(e_d[:], sc[:], AF.Exp)
                                e_n = ep.tile([128, 1024], f32r, tag="en",
                                              name=f"en{p}_{qc}_{i}")
                                nc.vector.tensor_mul(
                                    e_n[:].rearrange("p (a b) -> p a b", a=2),
                                    e_d[:].rearrange("p (a b) -> p a b", a=2),
                                    m1T[i][:, qs].unsqueeze(1).to_broadcast(
                                        [128, 2, 512]))
                                nc.tensor.matmul(
                                    psA[0:64, :],
                                    vpack[i][:, hA * 64:hA * 64 + 64],
                                    e_n[:, 0:512], start=(i == 0),
                                    stop=(i == NP - 1), skip_group_check=True)
                                nc.tensor.matmul(
                                    psB[0:64, :],
                                    vpack[i][:, hB * 64:hB * 64 + 64],
                                    e_n[:, 512:1024], start=(i == 0),
                                    stop=(i == NP - 1), skip_group_check=True)
                                nc.tensor.matmul(
                                    dAB[0:64, :], ones64[:], e_d[:, 0:512],
                                    start=(i == 0), stop=(i == NP - 1),
                                    skip_group_check=True)
                                nc.tensor.matmul(
                                    dAB[64:128, :], ones64[:], e_d[:, 512:1024],
                                    start=(i == 0), stop=(i == NP - 1),
                                    skip_group_check=True)
                        if structured:
                            dcpA = wk.tile([128, 512], f32, tag="dcp",
                                           name=f"dcpA{p}{qc}")
                            nc.vector.tensor_copy(dcpA[64:128, :],
                                                  psA[64:128, :])
                            dshA = wk.tile([128, 512], f32, tag="rAs",
                                           name=f"dshA{p}{qc}")
                            nc.sync.dma_start(dshA[0:64, :], dcpA[64:128, :])
                            rA = wk.tile([128, 512], f32, tag="rA",
                                         name=f"rA{p}{qc}")
                            nc.vector.reciprocal_approx_fast(
                                rA[0:64, :], dshA[0:64, :])
                            nc.vector.tensor_mul(dshA[0:64, :], rA[0:64, :],
                                                 vbq[0:64, qs])
                            nc.vector.tensor_mul(aot[p][0:64, qs], psA[0:64, :],
                                                 dshA[0:64, :])
                            dcpB = wk.tile([128, 512], f32, tag="dcp",
                                           name=f"dcpB{p}{qc}")
                            nc.vector.tensor_copy(dcpB[0:64, :], psB[0:64, :])
                            rB = wk.tile([128, 512], f32, tag="rB",
                                         name=f"rB{p}{qc}")
                            nc.vector.reciprocal_approx_fast(
                                rB[0:64, :], dcpB[0:64, :])
                            nc.vector.tensor_mul(dcpB[0:64, :], rB[0:64, :],
                                                 vbq[0:64, qs])
                            rBs = wk.tile([128, 512], f32, tag="rBs",
                                          name=f"rBs{p}{qc}")
                            nc.sync.dma_start(rBs[64:128, :], dcpB[0:64, :])
                            nc.vector.tensor_mul(aot[p][64:128, qs],
                                                 psB[64:128, :],
                                                 rBs[64:128, :])
                        else:
                            dcpG = wk.tile([128, 512], f32, tag="dcp",
                                           name=f"dcpG{p}{qc}")
                            nc.vector.tensor_copy(dcpG[0:64, :], dAB[0:64, :])
                            nc.sync.dma_start(dcpG[64:128, :], dAB[64:128, :])
                            rA = wk.tile([128, 512], f32, tag="rA",
                                         name=f"rA{p}{qc}")
                            nc.vector.reciprocal_approx_fast(
                                rA[0:64, :], dcpG[0:64, :])
                            nc.vector.tensor_mul(aot[p][0:64, qs],
                                                 psA[0:64, :], rA[0:64, :])
                            rBs = wk.tile([128, 512], f32, tag="rBs",
                                          name=f"rBs{p}{qc}")
                            nc.sync.dma_start(rBs[0:64, :], dcpG[64:128, :])
                            rGB = wk.tile([128, 512], f32, tag="rB",
                                          name=f"rGB{p}{qc}")
                            nc.vector.reciprocal_approx_fast(
                                rGB[0:64, :], rBs[0:64, :])
                            aoB = wk.tile([64, 512], f32r, tag="aoB",
                                          name=f"aoB{p}{qc}")
                            nc.vector.tensor_mul(aoB[:], psB[0:64, :],
                                                 rGB[0:64, :])
                            nc.sync.dma_start(aot[p][64:128, qs], aoB[:])

        # ---------------- PHASE C: output projection ----------------
        with tc.tile_pool(name="wo", bufs=1) as wop, \
             tc.tile_pool(name="yst", bufs=3) as yp, \
             tc.tile_pool(name="pyp", bufs=4, space="PSUM") as pyp:
            wot = []
            for j in range(NP):
                t = wop.tile([128, D], f32r, tag=f"wo{j}", name=f"wo{j}")
                nc.sync.dma_start(
                    t[:], wo_d.ap()[j * 128:(j + 1) * 128, :].bitcast(f32r))
                wot.append(t)
            for tq in range(NP):
                for mc in range(2):
                    ms = slice(mc * 512, (mc + 1) * 512)
                    ps = pyp.tile([128, 512], f32, tag="py",
                                  name=f"py{tq}_{mc}")
                    for j in range(NP):
                        nc.tensor.matmul(
                            ps[:], aot[j][:, tq * 128:(tq + 1) * 128],
                            wot[j][:, ms], start=(j == 0), stop=(j == NP - 1))
                    ys = yp.tile([128, 512], f32, tag="ys", name=f"ys{tq}_{mc}")
                    if structured:
                        nc.vector.tensor_scalar(
                            out=ys[:], in0=ps[:],
                            scalar1=vq8[:, tq:tq + 1], scalar2=None,
                            op0=ALU.mult)
                        nc.vector.tensor_add(ys[:], ys[:], bob[:, ms])
                    else:
                        nc.vector.tensor_add(ys[:], ps[:], bob[:, ms])
                    nc.sync.dma_start(
                        y_d.ap()[tq * 128:(tq + 1) * 128, ms], ys[:])
    nc.compile()
    return nc


_CACHE = {}


def _get(structured: bool):
    if structured not in _CACHE:
        _CACHE[structured] = _build(structured)
    return _CACHE[structured]


def kernel(query, key, value, mask0, mask1, Wq, bq, Wk, bk, Wv, bv, Wo, bo):
    query = np.asarray(query, dtype=np.float32)
    key = np.asarray(key, dtype=np.float32)
    value = np.asarray(value, dtype=np.float32)
    mask0 = np.asarray(mask0)
    mask1 = np.asarray(mask1)
    Wq = np.asarray(Wq, dtype=np.float32)
    bq = np.asarray(bq, dtype=np.float32)
    Wk = np.asarray(Wk, dtype=np.float32)
    bk = np.asarray(bk, dtype=np.float32)
    Wv = np.asarray(Wv, dtype=np.float32)
    bv = np.asarray(bv, dtype=np.float32)
    Wo = np.asarray(Wo, dtype=np.float32)
    bo = np.asarray(bo, dtype=np.float32)

    structured = True
    valids = []
    for b in range(B):
        v0 = mask0[b, 0, :]
        if not (mask0[b] == v0[None, :]).all() or not (
            mask1[b] == (v0[:, None] * v0[None, :])
        ).all():
            structured = False
            break
        valids.append(v0.astype(np.float32))

    nc = _get(structured)

    base = {
        "w2q": np.ascontiguousarray(Wq.transpose(1, 0, 2).reshape(D, D)),
        "w2k": np.ascontiguousarray(Wk.transpose(1, 0, 2).reshape(D, D)),
        "w2v": np.ascontiguousarray(Wv.transpose(1, 0, 2).reshape(D, D)),
        "wo": np.ascontiguousarray(Wo),
        "bq8": np.ascontiguousarray(bq.reshape(D) / 8.0),
        "bk": np.ascontiguousarray(bk.reshape(D)),
        "bv": np.ascontiguousarray(bv.reshape(D)),
        "bo_bc": np.broadcast_to(bo, (128, D)).copy(),
        "onescol": np.ones((128, 8 * 64), np.float32),
        "zeros": np.zeros((64, S), np.float32),
    }
    if not structured:
        base["eye"] = np.eye(128).astype(ml_dtypes.bfloat16)

    in_maps = []
    for b in range(B):
        m = dict(base)
        m["xqt"] = np.ascontiguousarray(query[b].T)
        m["xkt"] = np.ascontiguousarray(key[b].T)
        m["xvt"] = np.ascontiguousarray(value[b].T)
        if structured:
            v = valids[b]
            m["expb"] = np.ascontiguousarray(
                (-30000.0 * (1.0 - v)).reshape(NP, 128).T)
            m["vbq"] = np.broadcast_to(v, (128, S)).copy()
            m["vq8"] = np.ascontiguousarray(v.reshape(NP, 128).T)
        else:
            m["mbT"] = (-30000.0 * (1.0 - mask0[b].T.astype(np.float32))
                        ).astype(ml_dtypes.bfloat16)
            m["m1T"] = np.ascontiguousarray(mask1[b].T.astype(np.float32))
        in_maps.append(m)

    trace = os.environ.get("BASS_KERNEL_TRACE", "") == "1"
    res = run_bass_kernel_spmd(nc, in_maps, list(range(8)), trace=trace)
    kernel.last_exec_time_ns = res.exec_time_ns
    out = np.stack([res.results[b]["y"] for b in range(B)], axis=0)
    return out.astype(np.float32)


kernel.last_exec_time_ns = None


# revision 12
# speedup vs baseline: 1.3179x; 1.3179x over previous
"""MultiHeadAttention Trainium2 kernel: B=8, S=1024, D=1024, H=16, DK=64.

Batch-parallel over 8 NeuronCores (one batch element per core, no
collectives). Per core, attention runs in a transposed layout
(scores^T[s,q]) so the attn@V contraction needs no on-device
transposes; all data marshaling (transposes, weight reshape, mask
preprocessing) happens host-side in numpy.

Two variants:
 - structured: mask0[b] == broadcast(valid), mask1[b] == outer(valid,valid)
   (what reference.setup_inputs produces). mask0 folds into the exp bias
   ([P,1] per s-tile), mask1's row factor folds into the final scale, and
   the softmax denominator rides as 64 `ones` columns inside the V lhsT.
 - general: arbitrary 0/1 masks. mask0 is added as a -30000 bias via an
   identity matmul into the scores PSUM; mask1 multiplies e_d on the DVE;
   denominators use separate ones-matmuls.
"""
import os
import numpy as np
import ml_dtypes

import concourse.bacc as bacc
import concourse.mybir as mybir
from concourse.tile import TileContext
from concourse.bass_utils import run_bass_kernel_spmd

B, S, D, H, DK = 8, 1024, 1024, 16, 64
NP = 8
f32 = mybir.dt.float32
f32r = mybir.dt.float32r
bf16 = mybir.dt.bfloat16
AF = mybir.ActivationFunctionType
ALU = mybir.AluOpType


def _build(structured: bool):
    nc = bacc.Bacc("TRN2", target_bir_lowering=False, debug=False, num_devices=8)
    dt_qk = f32r if structured else bf16

    xqt_d = nc.dram_tensor("xqt", [D, S], f32, kind="ExternalInput")
    xkt_d = nc.dram_tensor("xkt", [D, S], f32, kind="ExternalInput")
    xvt_d = nc.dram_tensor("xvt", [D, S], f32, kind="ExternalInput")
    w2q_d = nc.dram_tensor("w2q", [D, D], f32, kind="ExternalInput")
    w2k_d = nc.dram_tensor("w2k", [D, D], f32, kind="ExternalInput")
    w2v_d = nc.dram_tensor("w2v", [D, D], f32, kind="ExternalInput")
    wo_d = nc.dram_tensor("wo", [D, D], f32, kind="ExternalInput")
    bq8_d = nc.dram_tensor("bq8", [D], f32, kind="ExternalInput")
    bk_d = nc.dram_tensor("bk", [D], f32, kind="ExternalInput")
    bv_d = nc.dram_tensor("bv", [D], f32, kind="ExternalInput")
    bo_d = nc.dram_tensor("bo_bc", [128, D], f32, kind="ExternalInput")
    onescol_d = nc.dram_tensor("onescol", [128, 8 * 64], f32, kind="ExternalInput")
    zeros_d = nc.dram_tensor("zeros", [64, S], f32, kind="ExternalInput")
    if structured:
        expb_d = nc.dram_tensor("expb", [128, NP], f32, kind="ExternalInput")
        vbq_d = nc.dram_tensor("vbq", [128, S], f32, kind="ExternalInput")
        vq8_d = nc.dram_tensor("vq8", [128, NP], f32, kind="ExternalInput")
    else:
        mbT_d = nc.dram_tensor("mbT", [S, S], bf16, kind="ExternalInput")
        m1T_d = nc.dram_tensor("m1T", [S, S], f32, kind="ExternalInput")
        eye_d = nc.dram_tensor("eye", [128, 128], bf16, kind="ExternalInput")
    y_d = nc.dram_tensor("y", [S, D], f32, kind="ExternalOutput")

    with TileContext(nc) as tc, \
         tc.tile_pool(name="persist", bufs=1) as app, \
         tc.tile_pool(name="vecs", bufs=1) as vp:
        aot = [app.tile([128, S], f32r, tag=f"aot{j}", name=f"aot{j}")
               for j in range(NP)]
        bq8v = vp.tile([128, NP], f32, name="bq8v")
        bkv = vp.tile([128, NP], f32, name="bkv")
        bvv = vp.tile([128, NP], f32, name="bvv")
        bob = vp.tile([128, D], f32, name="bob")
        nc.sync.dma_start(bq8v[:], bq8_d.ap().rearrange("(t p) -> p t", p=128))
        nc.sync.dma_start(bkv[:], bk_d.ap().rearrange("(t p) -> p t", p=128))
        nc.sync.dma_start(bvv[:], bv_d.ap().rearrange("(t p) -> p t", p=128))
        nc.sync.dma_start(bob[:], bo_d.ap())
        onesrow = vp.tile([1, 128], f32r, name="onesrow")
        bvr = vp.tile([1, D], f32r, name="bvr")
        nc.sync.dma_start(onesrow[:], onescol_d.ap()[0:1, 0:128].bitcast(f32r))
        nc.sync.dma_start(bvr[:],
                          bv_d.ap().rearrange("(a d) -> a d", a=1).bitcast(f32r))
        if structured:
            expb = vp.tile([128, NP], f32, name="expb")
            vbq = vp.tile([128, S], f32, name="vbq")
            vq8 = vp.tile([128, NP], f32, name="vq8")
            nc.sync.dma_start(expb[:], expb_d.ap())
            nc.sync.dma_start(vbq[:], vbq_d.ap())
            nc.sync.dma_start(vq8[:], vq8_d.ap())

        with tc.tile_pool(name="qkv", bufs=1) as pp:
            qtz = [pp.tile([128, S], dt_qk, tag=f"qtz{h}", name=f"qtz{h}")
                   for h in range(H)]
            kt = [pp.tile([128, S], dt_qk, tag=f"kt{p}", name=f"kt{p}")
                  for p in range(NP)]
            if structured:
                vpack = [pp.tile([128, NP, 192], f32r, tag=f"vp{j}",
                                 name=f"vp{j}") for j in range(NP)]
            else:
                vpack = [pp.tile([128, S], f32r, tag=f"vp{j}", name=f"vp{j}")
                         for j in range(NP)]
                ones64 = pp.tile([128, 64], f32r, name="ones64")
                nc.sync.dma_start(ones64[:],
                                  onescol_d.ap()[:, 0:64].bitcast(f32r))
                mbT = [pp.tile([128, S], bf16, tag=f"mbT{j}", name=f"mbT{j}")
                       for j in range(NP)]
                m1T = [pp.tile([128, S], f32, tag=f"m1T{j}", name=f"m1T{j}")
                       for j in range(NP)]
                eye = pp.tile([128, 128], bf16, name="eye")
                nc.sync.dma_start(eye[:], eye_d.ap())
                for j in range(NP):
                    nc.sync.dma_start(mbT[j][:],
                                      mbT_d.ap()[j * 128:(j + 1) * 128, :])
                    nc.sync.dma_start(m1T[j][:],
                                      m1T_d.ap()[j * 128:(j + 1) * 128, :])

            # ---------------- PHASE A: projections ----------------
            with tc.tile_pool(name="xt", bufs=2) as xp, \
                 tc.tile_pool(name="wst", bufs=6) as wp, \
                 tc.tile_pool(name="wvp", bufs=2) as wvp, \
                 tc.tile_pool(name="pjp", bufs=1, space="PSUM") as pjp:
                for which, (xd, wd) in enumerate(
                    [(xqt_d, w2q_d), (xkt_d, w2k_d), (xvt_d, w2v_d)]
                ):
                    for half in range(2):
                        js = list(range(half * 4, half * 4 + 4))
                        pss = {}
                        for j in js:
                            pss[j] = pjp.tile([128, S], f32, tag=f"pj{j % 4}",
                                              name=f"pj{which}_{j}")
                        for i in range(NP):
                            x = xp.tile([128, S], f32r, tag="x",
                                        name=f"x{which}_{half}_{i}")
                            nc.sync.dma_start(
                                x[:],
                                xd.ap()[i * 128:(i + 1) * 128, :].bitcast(f32r))
                            if which == 2:
                                # V natural [s, n]: lhsT = XvT d-tile sliced
                                # per s-tile, rhs = W2v row-tile
                                wv = wvp.tile([128, S], f32r, tag="wv",
                                             name=f"wv{half}_{i}")
                                nc.sync.dma_start(
                                    wv[:],
                                    wd.ap()[i * 128:(i + 1) * 128,
                                            :].bitcast(f32r))
                                for j in js:
                                    for c in range(2):
                                        nc.tensor.matmul(
                                            pss[j][:, c * 512:(c + 1) * 512],
                                            x[:, j * 128:(j + 1) * 128],
                                            wv[:, c * 512:(c + 1) * 512],
                                            start=(i == 0), stop=False)
                                if i == NP - 1:
                                    for j in js:
                                        for c in range(2):
                                            nc.tensor.matmul(
                                                pss[j][:, c * 512:(c + 1) * 512],
                                                onesrow[:],
                                                bvr[:, c * 512:(c + 1) * 512],
                                                start=False, stop=True)
                                continue
                            for j in js:
                                w = wp.tile([128, 128], f32r, tag="w",
                                            name=f"w{which}_{j}_{i}")
                                nc.sync.dma_start(
                                    w[:],
                                    wd.ap()[i * 128:(i + 1) * 128,
                                            j * 128:(j + 1) * 128].bitcast(f32r))
                                for c in range(2):
                                    nc.tensor.matmul(
                                        pss[j][:, c * 512:(c + 1) * 512], w[:],
                                        x[:, c * 512:(c + 1) * 512],
                                        start=(i == 0), stop=(i == NP - 1))
                        for j in js:
                            ps = pss[j]
                            if which == 0:
                                nc.vector.tensor_scalar(
                                    out=qtz[2 * j][0:64, :], in0=ps[0:64, :],
                                    scalar1=0.125, scalar2=bq8v[0:64, j:j + 1],
                                    op0=ALU.mult, op1=ALU.add)
                                nc.vector.tensor_scalar(
                                    out=qtz[2 * j + 1][64:128, :],
                                    in0=ps[64:128, :],
                                    scalar1=0.125, scalar2=bq8v[64:128, j:j + 1],
                                    op0=ALU.mult, op1=ALU.add)
                                nc.sync.dma_start(
                                    qtz[2 * j][64:128, :],
                                    zeros_d.ap().bitcast(f32r))
                                nc.sync.dma_start(
                                    qtz[2 * j + 1][0:64, :],
                                    zeros_d.ap().bitcast(f32r))
                            elif which == 1:
                                nc.vector.tensor_scalar(
                                    out=kt[j][:], in0=ps[:],
                                    scalar1=bkv[:, j:j + 1], scalar2=None,
                                    op0=ALU.add)
                            else:
                                if structured:
                                    nc.vector.tensor_copy(
                                        vpack[j][:, :, 0:64],
                                        ps[:].rearrange(
                                            "p (a two c) -> p a two c",
                                            two=2, c=64)[:, :, 0, :])
                                    nc.vector.tensor_copy(
                                        vpack[j][:, :, 128:192],
                                        ps[:].rearrange(
                                            "p (a two c) -> p a two c",
                                            two=2, c=64)[:, :, 1, :])
                                    nc.sync.dma_start(
                                        vpack[j][:, :, 64:128],
                                        onescol_d.ap().rearrange(
                                            "p (a c) -> p a c",
                                            c=64).bitcast(f32r))
                                else:
                                    nc.vector.tensor_copy(vpack[j][:], ps[:])

            # ---------------- PHASE B: attention ----------------
            with tc.tile_pool(name="ed", bufs=2) as ep, \
                 tc.tile_pool(name="epi", bufs=1) as wk, \
                 tc.tile_pool(name="scp", bufs=2, space="PSUM") as scp, \
                 tc.tile_pool(name="nvp", bufs=2 if structured else 1, space="PSUM") as nvp:
                for p in range(NP):
                    hA, hB = 2 * p, 2 * p + 1
                    for qc in range(2):
                        qs = slice(qc * 512, (qc + 1) * 512)
                        psA = nvp.tile([128, 512], f32, tag="psA",
                                       name=f"psA{p}_{qc}")
                        psB = nvp.tile([128, 512], f32, tag="psB",
                                       name=f"psB{p}_{qc}")
                        if not structured:
                            dAB = nvp.tile([128, 512], f32, tag="dAB",
                                           name=f"dAB{p}_{qc}")
                        for i in range(NP):
                            sc = scp.tile([128, 1024], f32, tag="sc",
                                          name=f"sc{p}_{qc}_{i}")
                            nc.tensor.matmul(
                                sc[:, 0:512], kt[p][:, i * 128:(i + 1) * 128],
                                qtz[hA][:, qs], start=True, stop=structured,
                                skip_group_check=True)
                            nc.tensor.matmul(
                                sc[:, 512:1024], kt[p][:, i * 128:(i + 1) * 128],
                                qtz[hB][:, qs], start=True, stop=structured,
                                skip_group_check=True)
                            if not structured:
                                nc.tensor.matmul(
                                    sc[:, 0:512], eye[:], mbT[i][:, qs],
                                    start=False, stop=True,
                                    skip_group_check=True)
                                nc.tensor.matmul(
                                    sc[:, 512:1024], eye[:], mbT[i][:, qs],
                                    start=False, stop=True,
                                    skip_group_check=True)
                            e_d = ep.tile([128, 1024], f32r, tag="ed",
                                          name=f"ed{p}_{qc}_{i}")
                            if structured:
                                nc.scalar.activation(
                                    e_d[:], sc[:], AF.Exp,
                                    bias=expb[:, i:i + 1], scale=1.0)
                                nc.tensor.matmul(
                                    psA[:], vpack[i][:, p, 0:128],
                                    e_d[:, 0:512],
                                    start=(i == 0), stop=(i == NP - 1),
                                    skip_group_check=True)
                                nc.tensor.matmul(
                                    psB[:], vpack[i][:, p, 64:192],
                                    e_d[:, 512:1024],
                                    start=(i == 0), stop=(i == NP - 1),
                                    skip_group_check=True)
                            else:
                                nc.scalar.activation(e_d[:], sc[:], AF.Exp)
                                e_n = ep.tile([128, 1024], f32r, tag="en",
                                              name=f"en{p}_{qc}_{i}")
                                nc.vector.tensor_mul(
                                    e_n[:].rearrange("p (a b) -> p a b", a=2),
                                    e_d[:].rearrange("p (a b) -> p a b", a=2),
                                    m1T[i][:, qs].unsqueeze(1).to_broadcast(
                                        [128, 2, 512]))
                                nc.tensor.matmul(
                                    psA[0:64, :],
                                    vpack[i][:, hA * 64:hA * 64 + 64],
                                    e_n[:, 0:512], start=(i == 0),
                                    stop=(i == NP - 1), skip_group_check=True)
                                nc.tensor.matmul(
                                    psB[0:64, :],
                                    vpack[i][:, hB * 64:hB * 64 + 64],
                                    e_n[:, 512:1024], start=(i == 0),
                                    stop=(i == NP - 1), skip_group_check=True)
                                nc.tensor.matmul(
                                    dAB[0:64, :], ones64[:], e_d[:, 0:512],
                                    start=(i == 0), stop=(i == NP - 1),
                                    skip_group_check=True)
                                nc.tensor.matmul(
                                    dAB[64:128, :], ones64[:], e_d[:, 512:1024],
                                    start=(i == 0), stop=(i == NP - 1),
                                    skip_group_check=True)
                        if structured:
                            dcpA = wk.tile([128, 512], f32, tag="dcp",
                                           name=f"dcpA{p}{qc}")
                            nc.vector.tensor_copy(dcpA[64:128, :],
                                                  psA[64:128, :])
                            dshA = wk.tile([128, 512], f32, tag="rAs",
                                           name=f"dshA{p}{qc}")
                            nc.sync.dma_start(dshA[0:64, :], dcpA[64:128, :])
                            rA = wk.tile([128, 512], f32, tag="rA",
                                         name=f"rA{p}{qc}")
                            nc.vector.reciprocal_approx_fast(
                                rA[0:64, :], dshA[0:64, :])
                            nc.vector.tensor_mul(dshA[0:64, :], rA[0:64, :],
                                                 vbq[0:64, qs])
                            nc.vector.tensor_mul(aot[p][0:64, qs], psA[0:64, :],
                                                 dshA[0:64, :])
                            dcpB = wk.tile([128, 512], f32, tag="dcp",
                                           name=f"dcpB{p}{qc}")
                            nc.vector.tensor_copy(dcpB[0:64, :], psB[0:64, :])
                            rB = wk.tile([128, 512], f32, tag="rB",
                                         name=f"rB{p}{qc}")
                            nc.vector.reciprocal_approx_fast(
                                rB[0:64, :], dcpB[0:64, :])
                            nc.vector.tensor_mul(dcpB[0:64, :], rB[0:64, :],
                                                 vbq[0:64, qs])
                            rBs = wk.tile([128, 512], f32, tag="rBs",
                                          name=f"rBs{p}{qc}")
                            nc.sync.dma_start(rBs[64:128, :], dcpB[0:64, :])
                            nc.vector.tensor_mul(aot[p][64:128, qs],
                                                 psB[64:128, :],
                                                 rBs[64:128, :])
                        else:
                            dcpG = wk.tile([128, 512], f32, tag="dcp",
                                           name=f"dcpG{p}{qc}")
                            nc.vector.tensor_copy(dcpG[0:64, :], dAB[0:64, :])
                            nc.sync.dma_start(dcpG[64:128, :], dAB[64:128, :])
                            rA = wk.tile([128, 512], f32, tag="rA",
                                         name=f"rA{p}{qc}")
                            nc.vector.reciprocal_approx_fast(
                                rA[0:64, :], dcpG[0:64, :])
                            nc.vector.tensor_mul(aot[p][0:64, qs],
                                                 psA[0:64, :], rA[0:64, :])
                            rBs = wk.tile([128, 512], f32, tag="rBs",
                                          name=f"rBs{p}{qc}")
                            nc.sync.dma_start(rBs[0:64, :], dcpG[64:128, :])
                            rGB = wk.tile([128, 512], f32, tag="rB",
                                          name=f"rGB{p}{qc}")
                            nc.vector.reciprocal_approx_fast(
                                rGB[0:64, :], rBs[0:64, :])
                            aoB = wk.tile([64, 512], f32r, tag="aoB",
                                          name=f"aoB{p}{qc}")
                            nc.vector.tensor_mul(aoB[:], psB[0:64, :],
                                                 rGB[0:64, :])
                            nc.sync.dma_start(aot[p][64:128, qs], aoB[:])

        # ---------------- PHASE C: output projection ----------------
        with tc.tile_pool(name="wo", bufs=1) as wop, \
             tc.tile_pool(name="yst", bufs=3) as yp, \
             tc.tile_pool(name="pyp", bufs=4, space="PSUM") as pyp:
            wot = []
            for j in range(NP):
                t = wop.tile([128, D], f32r, tag=f"wo{j}", name=f"wo{j}")
                nc.sync.dma_start(
                    t[:], wo_d.ap()[j * 128:(j + 1) * 128, :].bitcast(f32r))
                wot.append(t)
            for tq in range(NP):
                for mc in range(2):
                    ms = slice(mc * 512, (mc + 1) * 512)
                    ps = pyp.tile([128, 512], f32, tag="py",
                                  name=f"py{tq}_{mc}")
                    for j in range(NP):
                        nc.tensor.matmul(
                            ps[:], aot[j][:, tq * 128:(tq + 1) * 128],
                            wot[j][:, ms], start=(j == 0), stop=(j == NP - 1))
                    ys = yp.tile([128, 512], f32, tag="ys", name=f"ys{tq}_{mc}")
                    if structured:
                        nc.vector.tensor_scalar(
                            out=ys[:], in0=ps[:],
                            scalar1=vq8[:, tq:tq + 1], scalar2=None,
                            op0=ALU.mult)
                        nc.vector.tensor_add(ys[:], ys[:], bob[:, ms])
                    else:
                        nc.vector.tensor_add(ys[:], ps[:], bob[:, ms])
                    nc.sync.dma_start(
                        y_d.ap()[tq * 128:(tq + 1) * 128, ms], ys[:])
    nc.compile()
    return nc


_CACHE = {}


def _get(structured: bool):
    if structured not in _CACHE:
        _CACHE[structured] = _build(structured)
    return _CACHE[structured]


def kernel(query, key, value, mask0, mask1, Wq, bq, Wk, bk, Wv, bv, Wo, bo):
    query = np.asarray(query, dtype=np.float32)
    key = np.asarray(key, dtype=np.float32)
    value = np.asarray(value, dtype=np.float32)
    mask0 = np.asarray(mask0)
    mask1 = np.asarray(mask1)
    Wq = np.asarray(Wq, dtype=np.float32)
    bq = np.asarray(bq, dtype=np.float32)
    Wk = np.asarray(Wk, dtype=np.float32)
    bk = np.asarray(bk, dtype=np.float32)
    Wv = np.asarray(Wv, dtype=np.float32)
    bv = np.asarray(bv, dtype=np.float32)
    Wo = np.asarray(Wo, dtype=np.float32)
    bo = np.asarray(bo, dtype=np.float32)

    structured = True
    valids = []
    for b in range(B):
        v0 = mask0[b, 0, :]
        if not (mask0[b] == v0[None, :]).all() or not (
            mask1[b] == (v0[:, None] * v0[None, :])
        ).all():
            structured = False
            break
        valids.append(v0.astype(np.float32))

    nc = _get(structured)

    base = {
        "w2q": np.ascontiguousarray(Wq.transpose(1, 0, 2).reshape(D, D)),
        "w2k": np.ascontiguousarray(Wk.transpose(1, 0, 2).reshape(D, D)),
        "w2v": np.ascontiguousarray(Wv.transpose(1, 0, 2).reshape(D, D)),
        "wo": np.ascontiguousarray(Wo),
        "bq8": np.ascontiguousarray(bq.reshape(D) / 8.0),
        "bk": np.ascontiguousarray(bk.reshape(D)),
        "bv": np.ascontiguousarray(bv.reshape(D)),
        "bo_bc": np.broadcast_to(bo, (128, D)).copy(),
        "onescol": np.ones((128, 8 * 64), np.float32),
        "zeros": np.zeros((64, S), np.float32),
    }
    if not structured:
        base["eye"] = np.eye(128).astype(ml_dtypes.bfloat16)

    in_maps = []
    for b in range(B):
        m = dict(base)
        m["xqt"] = np.ascontiguousarray(query[b].T)
        m["xkt"] = np.ascontiguousarray(key[b].T)
        m["xvt"] = np.ascontiguousarray(value[b].T)
        if structured:
            v = valids[b]
            m["expb"] = np.ascontiguousarray(
                (-30000.0 * (1.0 - v)).reshape(NP, 128).T)
            m["vbq"] = np.broadcast_to(v, (128, S)).copy()
            m["vq8"] = np.ascontiguousarray(v.reshape(NP, 128).T)
        else:
            m["mbT"] = (-30000.0 * (1.0 - mask0[b].T.astype(np.float32))
                        ).astype(ml_dtypes.bfloat16)
            m["m1T"] = np.ascontiguousarray(mask1[b].T.astype(np.float32))
        in_maps.append(m)

    trace = os.environ.get("BASS_KERNEL_TRACE", "") == "1"
    res = run_bass_kernel_spmd(nc, in_maps, list(range(8)), trace=trace)
    kernel.last_exec_time_ns = res.exec_time_ns
    out = np.stack([res.results[b]["y"] for b in range(B)], axis=0)
    return out.astype(np.float32)


kernel.last_exec_time_ns = None


# revision 15
# speedup vs baseline: 1.3760x; 1.0441x over previous
"""MultiHeadAttention Trainium2 kernel: B=8, S=1024, D=1024, H=16, DK=64.

Batch-parallel over 8 NeuronCores (one batch element per core, no
collectives). Per core, attention runs in a transposed layout
(scores^T[s,q]) so the attn@V contraction needs no on-device
transposes; all data marshaling (transposes, weight reshape, mask
preprocessing) happens host-side in numpy.

Two variants:
 - structured: mask0[b] == broadcast(valid), mask1[b] == outer(valid,valid)
   (what reference.setup_inputs produces). mask0 folds into the exp bias
   ([P,1] per s-tile), mask1's row factor folds into the final scale, and
   the softmax denominator rides as 64 `ones` columns inside the V lhsT.
 - general: arbitrary 0/1 masks. mask0 is added as a -30000 bias via an
   identity matmul into the scores PSUM; mask1 multiplies e_d on the DVE;
   denominators use separate ones-matmuls.
"""
import os
import numpy as np
import ml_dtypes

import concourse.bacc as bacc
import concourse.mybir as mybir
from concourse.tile import TileContext
from concourse.bass_utils import run_bass_kernel_spmd

B, S, D, H, DK = 8, 1024, 1024, 16, 64
NP = 8
f32 = mybir.dt.float32
f32r = mybir.dt.float32r
bf16 = mybir.dt.bfloat16
AF = mybir.ActivationFunctionType
ALU = mybir.AluOpType


def _build(structured: bool):
    nc = bacc.Bacc("TRN2", target_bir_lowering=False, debug=False, num_devices=8)
    dt_qk = f32r if structured else bf16

    xqt_d = nc.dram_tensor("xqt", [D, S], f32, kind="ExternalInput")
    xkt_d = nc.dram_tensor("xkt", [D, S], f32, kind="ExternalInput")
    xvt_d = nc.dram_tensor("xvt", [D, S], f32, kind="ExternalInput")
    w2q_d = nc.dram_tensor("w2q", [D, D], f32, kind="ExternalInput")
    w2k_d = nc.dram_tensor("w2k", [D, D], f32, kind="ExternalInput")
    w2v_d = nc.dram_tensor("w2v", [D, D], f32, kind="ExternalInput")
    wo_d = nc.dram_tensor("wo", [D, D], f32, kind="ExternalInput")
    bq8_d = nc.dram_tensor("bq8", [D], f32, kind="ExternalInput")
    bk_d = nc.dram_tensor("bk", [D], f32, kind="ExternalInput")
    bv_d = nc.dram_tensor("bv", [D], f32, kind="ExternalInput")
    bo_d = nc.dram_tensor("bo_bc", [128, D], f32, kind="ExternalInput")
    onescol_d = nc.dram_tensor("onescol", [128, 8 * 64], f32, kind="ExternalInput")
    zeros_d = nc.dram_tensor("zeros", [64, S],
                            f32 if structured else bf16, kind="ExternalInput")
    if structured:
        expb_d = nc.dram_tensor("expb", [128, NP], f32, kind="ExternalInput")
        vbq_d = nc.dram_tensor("vbq", [128, S], f32, kind="ExternalInput")
        vq8_d = nc.dram_tensor("vq8", [128, NP], f32, kind="ExternalInput")
    else:
        mbT_d = nc.dram_tensor("mbT", [S, S], bf16, kind="ExternalInput")
        m1T_d = nc.dram_tensor("m1T", [S, S], f32, kind="ExternalInput")
        eye_d = nc.dram_tensor("eye", [128, 128], bf16, kind="ExternalInput")
    y_d = nc.dram_tensor("y", [S, D], f32, kind="ExternalOutput")

    with TileContext(nc) as tc, \
         tc.tile_pool(name="persist", bufs=1) as app, \
         tc.tile_pool(name="vecs", bufs=1) as vp:
        aot = [app.tile([128, S], f32r, tag=f"aot{j}", name=f"aot{j}")
               for j in range(NP)]
        bq8v = vp.tile([128, NP], f32, name="bq8v")
        bkv = vp.tile([128, NP], f32, name="bkv")
        bvv = vp.tile([128, NP], f32, name="bvv")
        bob = vp.tile([128, D], f32, name="bob")
        nc.sync.dma_start(bq8v[:], bq8_d.ap().rearrange("(t p) -> p t", p=128))
        nc.sync.dma_start(bkv[:], bk_d.ap().rearrange("(t p) -> p t", p=128))
        nc.sync.dma_start(bvv[:], bv_d.ap().rearrange("(t p) -> p t", p=128))
        nc.sync.dma_start(bob[:], bo_d.ap())
        onesrow = vp.tile([1, 128], f32r, name="onesrow")
        bvr = vp.tile([1, D], f32r, name="bvr")
        nc.sync.dma_start(onesrow[:], onescol_d.ap()[0:1, 0:128].bitcast(f32r))
        nc.sync.dma_start(bvr[:],
                          bv_d.ap().rearrange("(a d) -> a d", a=1).bitcast(f32r))
        if structured:
            expb = vp.tile([128, NP], f32, name="expb")
            vbq = vp.tile([128, S], f32, name="vbq")
            vq8 = vp.tile([128, NP], f32, name="vq8")
            nc.sync.dma_start(expb[:], expb_d.ap())
            nc.sync.dma_start(vbq[:], vbq_d.ap())
            nc.sync.dma_start(vq8[:], vq8_d.ap())

        with tc.tile_pool(name="qkv", bufs=1) as pp:
            qtz = [pp.tile([128, S], dt_qk, tag=f"qtz{h}", name=f"qtz{h}")
                   for h in range(H)]
            kt = [pp.tile([128, S], dt_qk, tag=f"kt{p}", name=f"kt{p}")
                  for p in range(NP)]
            if structured:
                vpack = [pp.tile([128, NP, 192], f32r, tag=f"vp{j}",
                                 name=f"vp{j}") for j in range(NP)]
            else:
                vpack = [pp.tile([128, S], f32r, tag=f"vp{j}", name=f"vp{j}")
                         for j in range(NP)]
                ones64 = pp.tile([128, 64], f32r, name="ones64")
                nc.sync.dma_start(ones64[:],
                                  onescol_d.ap()[:, 0:64].bitcast(f32r))
                mbT = [pp.tile([128, S], bf16, tag=f"mbT{j}", name=f"mbT{j}")
                       for j in range(NP)]
                m1T = [pp.tile([128, S], f32, tag=f"m1T{j}", name=f"m1T{j}")
                       for j in range(NP)]
                eye = pp.tile([128, 128], bf16, name="eye")
                nc.sync.dma_start(eye[:], eye_d.ap())
                for j in range(NP):
                    nc.sync.dma_start(mbT[j][:],
                                      mbT_d.ap()[j * 128:(j + 1) * 128, :])
                    nc.sync.dma_start(m1T[j][:],
                                      m1T_d.ap()[j * 128:(j + 1) * 128, :])

            # ---------------- PHASE A: projections ----------------
            with tc.tile_pool(name="xt", bufs=2) as xp, \
                 tc.tile_pool(name="wst", bufs=6) as wp, \
                 tc.tile_pool(name="wvp", bufs=2) as wvp, \
                 tc.tile_pool(name="pjp", bufs=1, space="PSUM") as pjp:
                for which, (xd, wd) in enumerate(
                    [(xqt_d, w2q_d), (xkt_d, w2k_d), (xvt_d, w2v_d)]
                ):
                    for half in range(2):
                        js = list(range(half * 4, half * 4 + 4))
                        pss = {}
                        for j in js:
                            pss[j] = pjp.tile([128, S], f32, tag=f"pj{j % 4}",
                                              name=f"pj{which}_{j}")
                        for i in range(NP):
                            x = xp.tile([128, S], f32r, tag="x",
                                        name=f"x{which}_{half}_{i}")
                            nc.sync.dma_start(
                                x[:],
                                xd.ap()[i * 128:(i + 1) * 128, :].bitcast(f32r))
                            if which == 2:
                                # V natural [s, n]: lhsT = XvT d-tile sliced
                                # per s-tile, rhs = W2v row-tile
                                wv = wvp.tile([128, S], f32r, tag="wv",
                                             name=f"wv{half}_{i}")
                                nc.sync.dma_start(
                                    wv[:],
                                    wd.ap()[i * 128:(i + 1) * 128,
                                            :].bitcast(f32r))
                                for j in js:
                                    for c in range(2):
                                        nc.tensor.matmul(
                                            pss[j][:, c * 512:(c + 1) * 512],
                                            x[:, j * 128:(j + 1) * 128],
                                            wv[:, c * 512:(c + 1) * 512],
                                            start=(i == 0), stop=False)
                                if i == NP - 1:
                                    for j in js:
                                        for c in range(2):
                                            nc.tensor.matmul(
                                                pss[j][:, c * 512:(c + 1) * 512],
                                                onesrow[:],
                                                bvr[:, c * 512:(c + 1) * 512],
                                                start=False, stop=True)
                                continue
                            for j in js:
                                w = wp.tile([128, 128], f32r, tag="w",
                                            name=f"w{which}_{j}_{i}")
                                nc.sync.dma_start(
                                    w[:],
                                    wd.ap()[i * 128:(i + 1) * 128,
                                            j * 128:(j + 1) * 128].bitcast(f32r))
                                for c in range(2):
                                    nc.tensor.matmul(
                                        pss[j][:, c * 512:(c + 1) * 512], w[:],
                                        x[:, c * 512:(c + 1) * 512],
                                        start=(i == 0), stop=(i == NP - 1))
                        for j in js:
                            ps = pss[j]
                            if which == 0:
                                nc.vector.tensor_scalar(
                                    out=qtz[2 * j][0:64, :], in0=ps[0:64, :],
                                    scalar1=0.125, scalar2=bq8v[0:64, j:j + 1],
                                    op0=ALU.mult, op1=ALU.add)
                                nc.vector.tensor_scalar(
                                    out=qtz[2 * j + 1][64:128, :],
                                    in0=ps[64:128, :],
                                    scalar1=0.125, scalar2=bq8v[64:128, j:j + 1],
                                    op0=ALU.mult, op1=ALU.add)
                                zap = (zeros_d.ap().bitcast(f32r)
                                       if structured else zeros_d.ap())
                                nc.sync.dma_start(qtz[2 * j][64:128, :], zap)
                                nc.sync.dma_start(qtz[2 * j + 1][0:64, :], zap)
                            elif which == 1:
                                nc.vector.tensor_scalar(
                                    out=kt[j][:], in0=ps[:],
                                    scalar1=bkv[:, j:j + 1], scalar2=None,
                                    op0=ALU.add)
                            else:
                                if structured:
                                    nc.vector.tensor_copy(
                                        vpack[j][:, :, 0:64],
                                        ps[:].rearrange(
                                            "p (a two c) -> p a two c",
                                            two=2, c=64)[:, :, 0, :])
                                    nc.vector.tensor_copy(
                                        vpack[j][:, :, 128:192],
                                        ps[:].rearrange(
                                            "p (a two c) -> p a two c",
                                            two=2, c=64)[:, :, 1, :])
                                    nc.sync.dma_start(
                                        vpack[j][:, :, 64:128],
                                        onescol_d.ap().rearrange(
                                            "p (a c) -> p a c",
                                            c=64).bitcast(f32r))
                                else:
                                    nc.vector.tensor_copy(vpack[j][:], ps[:])

            # ---------------- PHASE B: attention ----------------
            with tc.tile_pool(name="ed", bufs=2) as ep, \
                 tc.tile_pool(name="epi", bufs=1) as wk, \
                 tc.tile_pool(name="scp", bufs=2, space="PSUM") as scp, \
                 tc.tile_pool(name="nvp", bufs=2 if structured else 1, space="PSUM") as nvp:
                for p in range(NP):
                    hA, hB = 2 * p, 2 * p + 1
                    for qc in range(2):
                        qs = slice(qc * 512, (qc + 1) * 512)
                        psA = nvp.tile([128, 512], f32, tag="psA",
                                       name=f"psA{p}_{qc}")
                        psB = nvp.tile([128, 512], f32, tag="psB",
                                       name=f"psB{p}_{qc}")
                        if not structured:
                            dA = nvp.tile([64, 512], f32, tag="dA",
                                          name=f"dA{p}_{qc}")
                            dB = nvp.tile([64, 512], f32, tag="dB",
                                          name=f"dB{p}_{qc}")
                        for i in range(NP):
                            sc = scp.tile([128, 1024], f32, tag="sc",
                                          name=f"sc{p}_{qc}_{i}")
                            nc.tensor.matmul(
                                sc[:, 0:512], kt[p][:, i * 128:(i + 1) * 128],
                                qtz[hA][:, qs], start=True, stop=structured,
                                skip_group_check=True)
                            nc.tensor.matmul(
                                sc[:, 512:1024], kt[p][:, i * 128:(i + 1) * 128],
                                qtz[hB][:, qs], start=True, stop=structured,
                                skip_group_check=True)
                            if not structured:
                                nc.tensor.matmul(
                                    sc[:, 0:512], eye[:], mbT[i][:, qs],
                                    start=False, stop=True,
                                    skip_group_check=True)
                                nc.tensor.matmul(
                                    sc[:, 512:1024], eye[:], mbT[i][:, qs],
                                    start=False, stop=True,
                                    skip_group_check=True)
                            e_d = ep.tile([128, 1024], f32r, tag="ed",
                                          name=f"ed{p}_{qc}_{i}")
                            if structured:
                                nc.scalar.activation(
                                    e_d[:], sc[:], AF.Exp,
                                    bias=expb[:, i:i + 1], scale=1.0)
                                nc.tensor.matmul(
                                    psA[:], vpack[i][:, p, 0:128],
                                    e_d[:, 0:512],
                                    start=(i == 0), stop=(i == NP - 1),
                                    skip_group_check=True)
                                nc.tensor.matmul(
                                    psB[:], vpack[i][:, p, 64:192],
                                    e_d[:, 512:1024],
                                    start=(i == 0), stop=(i == NP - 1),
                                    skip_group_check=True)
                            else:
                                nc.scalar.activation(e_d[:], sc[:], AF.Exp)
                                e_n = ep.tile([128, 1024], f32r, tag="en",
                                              name=f"en{p}_{qc}_{i}")
                                nc.vector.tensor_mul(
                                    e_n[:].rearrange("p (a b) -> p a b", a=2),
                                    e_d[:].rearrange("p (a b) -> p a b", a=2),
                                    m1T[i][:, qs].unsqueeze(1).to_broadcast(
                                        [128, 2, 512]))
                                nc.tensor.matmul(
                                    psA[0:64, :],
                                    vpack[i][:, hA * 64:hA * 64 + 64],
                                    e_n[:, 0:512], start=(i == 0),
                                    stop=(i == NP - 1), skip_group_check=True)
                                nc.tensor.matmul(
                                    psB[0:64, :],
                                    vpack[i][:, hB * 64:hB * 64 + 64],
                                    e_n[:, 512:1024], start=(i == 0),
                                    stop=(i == NP - 1), skip_group_check=True)
                                nc.tensor.matmul(
                                    dA[:], ones64[:], e_d[:, 0:512],
                                    start=(i == 0), stop=(i == NP - 1),
                                    skip_group_check=True)
                                nc.tensor.matmul(
                                    dB[:], ones64[:], e_d[:, 512:1024],
                                    start=(i == 0), stop=(i == NP - 1),
                                    skip_group_check=True)
                        if structured:
                            dcpA = wk.tile([128, 512], f32, tag="dcp",
                                           name=f"dcpA{p}{qc}")
                            nc.vector.tensor_copy(dcpA[64:128, :],
                                                  psA[64:128, :])
                            dshA = wk.tile([128, 512], f32, tag="rAs",
                                           name=f"dshA{p}{qc}")
                            nc.sync.dma_start(dshA[0:64, :], dcpA[64:128, :])
                            rA = wk.tile([128, 512], f32, tag="rA",
                                         name=f"rA{p}{qc}")
                            nc.vector.reciprocal_approx_fast(
                                rA[0:64, :], dshA[0:64, :])
                            nc.vector.tensor_mul(dshA[0:64, :], rA[0:64, :],
                                                 vbq[0:64, qs])
                            nc.vector.tensor_mul(aot[p][0:64, qs], psA[0:64, :],
                                                 dshA[0:64, :])
                            dcpB = wk.tile([128, 512], f32, tag="dcp",
                                           name=f"dcpB{p}{qc}")
                            nc.vector.tensor_copy(dcpB[0:64, :], psB[0:64, :])
                            rB = wk.tile([128, 512], f32, tag="rB",
                                         name=f"rB{p}{qc}")
                            nc.vector.reciprocal_approx_fast(
                                rB[0:64, :], dcpB[0:64, :])
                            nc.vector.tensor_mul(dcpB[0:64, :], rB[0:64, :],
                                                 vbq[0:64, qs])
                            rBs = wk.tile([128, 512], f32, tag="rBs",
                                          name=f"rBs{p}{qc}")
                            nc.sync.dma_start(rBs[64:128, :], dcpB[0:64, :])
                            nc.vector.tensor_mul(aot[p][64:128, qs],
                                                 psB[64:128, :],
                                                 rBs[64:128, :])
                        else:
                            dcpG = wk.tile([128, 512], f32, tag="dcp",
                                           name=f"dcpG{p}{qc}")
                            nc.vector.tensor_copy(dcpG[0:64, :], dA[:])
                            rA = wk.tile([128, 512], f32, tag="rA",
                                         name=f"rA{p}{qc}")
                            nc.vector.reciprocal_approx_fast(
                                rA[0:64, :], dcpG[0:64, :])
                            nc.vector.tensor_mul(aot[p][0:64, qs],
                                                 psA[0:64, :], rA[0:64, :])
                            rBs = wk.tile([128, 512], f32, tag="rBs",
                                          name=f"rBs{p}{qc}")
                            nc.vector.tensor_copy(rBs[0:64, :], dB[:])
                            rGB = wk.tile([128, 512], f32, tag="rB",
                                          name=f"rGB{p}{qc}")
                            nc.vector.reciprocal_approx_fast(
                                rGB[0:64, :], rBs[0:64, :])
                            aoB = wk.tile([64, 512], f32r, tag="aoB",
                                          name=f"aoB{p}{qc}")
                            nc.vector.tensor_mul(aoB[:], psB[0:64, :],
                                                 rGB[0:64, :])
                            nc.sync.dma_start(aot[p][64:128, qs], aoB[:])

        # ---------------- PHASE C: output projection ----------------
        with tc.tile_pool(name="wo", bufs=1) as wop, \
             tc.tile_pool(name="yst", bufs=3) as yp, \
             tc.tile_pool(name="pyp", bufs=4, space="PSUM") as pyp:
            wot = []
            for j in range(NP):
                t = wop.tile([128, D], f32r, tag=f"wo{j}", name=f"wo{j}")
                nc.sync.dma_start(
                    t[:], wo_d.ap()[j * 128:(j + 1) * 128, :].bitcast(f32r))
                wot.append(t)
            for tq in range(NP):
                for mc in range(2):
                    ms = slice(mc * 512, (mc + 1) * 512)
                    ps = pyp.tile([128, 512], f32, tag="py",
                                  name=f"py{tq}_{mc}")
                    for j in range(NP):
                        nc.tensor.matmul(
                            ps[:], aot[j][:, tq * 128:(tq + 1) * 128],
                            wot[j][:, ms], start=(j == 0), stop=(j == NP - 1))
                    ys = yp.tile([128, 512], f32, tag="ys", name=f"ys{tq}_{mc}")
                    if structured:
                        nc.vector.tensor_scalar(
                            out=ys[:], in0=ps[:],
                            scalar1=vq8[:, tq:tq + 1], scalar2=None,
                            op0=ALU.mult)
                        nc.vector.tensor_add(ys[:], ys[:], bob[:, ms])
                    else:
                        nc.vector.tensor_add(ys[:], ps[:], bob[:, ms])
                    nc.sync.dma_start(
                        y_d.ap()[tq * 128:(tq + 1) * 128, ms], ys[:])
    nc.compile()
    return nc


_CACHE = {}


def _get(structured: bool):
    if structured not in _CACHE:
        _CACHE[structured] = _build(structured)
    return _CACHE[structured]


def kernel(query, key, value, mask0, mask1, Wq, bq, Wk, bk, Wv, bv, Wo, bo):
    query = np.asarray(query, dtype=np.float32)
    key = np.asarray(key, dtype=np.float32)
    value = np.asarray(value, dtype=np.float32)
    mask0 = np.asarray(mask0)
    mask1 = np.asarray(mask1)
    Wq = np.asarray(Wq, dtype=np.float32)
    bq = np.asarray(bq, dtype=np.float32)
    Wk = np.asarray(Wk, dtype=np.float32)
    bk = np.asarray(bk, dtype=np.float32)
    Wv = np.asarray(Wv, dtype=np.float32)
    bv = np.asarray(bv, dtype=np.float32)
    Wo = np.asarray(Wo, dtype=np.float32)
    bo = np.asarray(bo, dtype=np.float32)

    structured = True
    valids = []
    for b in range(B):
        v0 = mask0[b, 0, :]
        if not (mask0[b] == v0[None, :]).all() or not (
            mask1[b] == (v0[:, None] * v0[None, :])
        ).all():
            structured = False
            break
        valids.append(v0.astype(np.float32))

    nc = _get(structured)

    base = {
        "w2q": np.ascontiguousarray(Wq.transpose(1, 0, 2).reshape(D, D)),
        "w2k": np.ascontiguousarray(Wk.transpose(1, 0, 2).reshape(D, D)),
        "w2v": np.ascontiguousarray(Wv.transpose(1, 0, 2).reshape(D, D)),
        "wo": np.ascontiguousarray(Wo),
        "bq8": np.ascontiguousarray(bq.reshape(D) / 8.0),
        "bk": np.ascontiguousarray(bk.reshape(D)),
        "bv": np.ascontiguousarray(bv.reshape(D)),
        "bo_bc": np.broadcast_to(bo, (128, D)).copy(),
        "onescol": np.ones((128, 8 * 64), np.float32),
        
    }
    if structured:
        base["zeros"] = np.zeros((64, S), np.float32)
    else:
        base["zeros"] = np.zeros((64, S), ml_dtypes.bfloat16)
        base["eye"] = np.eye(128).astype(ml_dtypes.bfloat16)

    in_maps = []
    for b in range(B):
        m = dict(base)
        m["xqt"] = np.ascontiguousarray(query[b].T)
        m["xkt"] = np.ascontiguousarray(key[b].T)
        m["xvt"] = np.ascontiguousarray(value[b].T)
        if structured:
            v = valids[b]
            m["expb"] = np.ascontiguousarray(
                (-30000.0 * (1.0 - v)).reshape(NP, 128).T)
            m["vbq"] = np.broadcast_to(v, (128, S)).copy()
            m["vq8"] = np.ascontiguousarray(v.reshape(NP, 128).T)
        else:
            m["mbT"] = (-30000.0 * (1.0 - mask0[b].T.astype(np.float32))
                        ).astype(ml_dtypes.bfloat16)
            m["m1T"] = np.ascontiguousarray(mask1[b].T.astype(np.float32))
        in_maps.append(m)

    trace = os.environ.get("BASS_KERNEL_TRACE", "") == "1"
    res = run_bass_kernel_spmd(nc, in_maps, list(range(8)), trace=trace)
    kernel.last_exec_time_ns = res.exec_time_ns
    out = np.stack([res.results[b]["y"] for b in range(B)], axis=0)
    return out.astype(np.float32)


kernel.last_exec_time_ns = None
